# revision 61
# baseline (speedup 1.0000x reference)
"""HadLinear Trainium2 kernel: out = blockwise_FWHT(x)/sqrt(1024) @ w.T.

Strategy (8 NeuronCores, tensor-parallel over output features):
  - out = x @ V with V = B @ w.T, B = blockdiag(H_1024, x4)/32.  V is
    computed on-device via the Kronecker split H_1024 = H_8 (x) H_128:
    stage A runs 32 DoubleRow PE matmuls T1 = (H_128/32) @ (w_hi+w_lo)
    (H entries +-2^-5 are exact in e4m3; w arrives as a host-split
    e4m3 hi/lo pair contracted in the two DoubleRow slots), then three
    H_8 butterfly stages as add/sub pairs across Pool/DVE, and an
    e4m3 hi/lo split of V (hi cast on ACT, lo on DVE / Pool+neg-hi).
  - The big matmul runs in fp8 (e4m3) DoubleRow perf mode, which
    contracts two 128-chunks per instruction at 0.5 PE cycles per
    output row (4x the bf16 MAC rate).  Precision is recovered with a
    first-order hi/lo decomposition:
        x*16  = x_hi + x_lo   (e4m3 pair, host-side split)
        V*64  = V_hi + V_lo   (e4m3 pair, on-device split of bf16 V)
        out   = [x_hi@V_hi + x_hi@V_lo + x_lo@V_hi] * 2^-10
    Per chunk pair {2k, 2k+1} this is 3 DoubleRow instructions, all
    with natural strides:
        I_main:    (x_hi[2k], x_hi[2k+1]) x (V_hi[2k], V_hi[2k+1])
        I_corr(j): (x_hi[j],  x_lo[j])    x (V_lo[j],  V_hi[j])
    The corrections are dropped for the last 6 of 32 chunks
    (DROP_CORR), trading rel err 0.005 -> 0.0162 (gate 2e-2) for
    ~41us of PE time.  Net PE floor ~292us vs bf16's 437us.
  - w is column-sharded: core c owns output features [c*512,(c+1)*512).
    Every core streams the full x (host-split fp8 hi/lo interleaved,
    feature-major tiles of 512 tokens) on the two HWDGE queues plus
    the Pool SWDGE queue; the scalar (ACT) queue carries almost no
    DMA because queued wire time blocks later ACT compute.
  - Stage-2 accumulators rotate over 6 PSUM banks with per-acc
    start/stop tracking, and the g0/g1 prologue is hand-interleaved
    so the in-order PE stream never head-of-line blocks on a V block
    stage A has not delivered yet.  fp32 PSUM accumulation; the 2^-10
    descale rides the ACT evacuation for free.
  Measured (CoreSim calibrated model): 333.0us vs 454.0us baseline;
  rel err 0.0162 (verified on hardware, deterministic seeded inputs).
"""

import numpy as np
import ml_dtypes

import concourse.bacc as bacc
import concourse.tile as tile
import concourse.mybir as mybir
from concourse.bass_utils import run_bass_kernel_spmd

N_CORES = 8
B, S, D = 4, 2048, 4096          # input (B, S, D)
TOK = B * S                      # 8192 tokens
BLOCK = 1024                     # Hadamard block
OUT_PER_CORE = D // N_CORES      # 512 output features per core
K_CHUNKS = D // 128              # 32 contraction chunks
QR = BLOCK // 128                # 8 chunks per Hadamard block
N_BLOCKS = D // BLOCK            # 4 Hadamard blocks
N_PAIRS = K_CHUNKS // 2          # 16 chunk pairs
G_TOK = 512                      # tokens per x tile
N_GROUPS = TOK // G_TOK          # 16 token groups
G_M = G_TOK // 128               # 4 output m-chunks per group

# Correction-dropped chunks: the hi*lo cross terms are skipped for these
# contraction chunks (numerics: rel err 0.0048 -> ~0.016, gate is 2e-2;
# saves 6 DoubleRow instructions per accumulator = ~41us of PE time).
DROP_CORR = frozenset((26, 27, 28, 29, 30, 31))
N_ACC_INSTR = 16 + 32 - len(DROP_CORR)   # mains + kept corrections

SX = 16.0                        # x prescale (host)
SV = 64.0                        # w prescale (host; V inherits it)
DESCALE = 1.0 / (SX * SV)        # exact power of 2, applied at evac

BF16 = ml_dtypes.bfloat16
E4M3 = ml_dtypes.float8_e4m3

_PROGRAM = None


def _h128_table():
    """H[p, q] = H_128[p, q] / 32, bf16 (exact: entries are +-2^-5)."""
    idx = np.arange(128)
    anded = idx[:, None] & idx[None, :]
    par = np.zeros_like(anded)
    v = anded
    while v.any():
        par ^= v & 1
        v >>= 1
    return ((1 - 2 * par).astype(np.float32) / 32.0).astype(BF16)


def _build_program():
    nc = bacc.Bacc("TRN2", target_bir_lowering=False, debug=False,
                   num_devices=N_CORES)
    # xg[g, blk, p, q, hl, t] = split(x[g*512 + t, blk*1024 + q*128 + p] * 16)
    #   hl: 0 = e4m3 hi, 1 = e4m3 residual lo
    x_d = nc.dram_tensor("xg8", [N_GROUPS, N_BLOCKS, 128, QR, 2, G_TOK],
                         mybir.dt.float8e4, kind="ExternalInput")
    # wt[blk, p, q2, q1, q0, hl, o]: e4m3 hi/lo split of
    # 64 * w[c*512 + o, blk*1024 + q*128 + p] (host-side, elementwise).
    # Stage A contracts both slots in one DoubleRow matmul: H entries
    # (+-2^-5) are exact in e4m3, so T1 = H @ (w_hi + w_lo) is computed
    # at half the PE cost and with ~4x less w-quantization error than
    # the bf16-w path.
    w_d = nc.dram_tensor("wt", [N_BLOCKS, 128, 2, 2, 2, 2, OUT_PER_CORE],
                         mybir.dt.float8e4, kind="ExternalInput")
    h_d = nc.dram_tensor("h", [128, 2, 128], mybir.dt.float8e4,
                         kind="ExternalInput")
    # out[g, t, ml, o] = out_full[g*512 + ml*128 + t, c*512 + o]
    o_d = nc.dram_tensor("out", [N_GROUPS, 128, G_M, OUT_PER_CORE],
                         mybir.dt.bfloat16, kind="ExternalOutput")


    with tile.TileContext(nc) as tc:
        with (
            tc.tile_pool(name="consts", bufs=1) as consts,
            tc.tile_pool(name="t1p", bufs=1) as t1p,
            tc.tile_pool(name="t2p", bufs=1) as t2p,
            tc.tile_pool(name="wsp", bufs=1) as wsp,
            tc.tile_pool(name="v8p", bufs=1) as v8p,
            tc.tile_pool(name="nhp", bufs=2) as nhp,
            tc.tile_pool(name="xin", bufs=2) as xin,
            tc.tile_pool(name="ost", bufs=2) as ost,
            tc.tile_pool(name="ps1", bufs=1, space="PSUM") as ps1,
            tc.tile_pool(name="ps2", bufs=1, space="PSUM") as ps2,
        ):
            h = consts.tile([128, 2, 128], mybir.dt.float8e4)
            nc.sync.dma_start(h[:], h_d[:])

            # v8[p, blk, q2, q1, q0, {lo,hi}, o]: e4m3 split of V*64;
            # chunk index kc = blk*8 + q2*4 + q1*2 + q0, so kc pairs are
            # q0-adjacent and all stage-2 APs below have natural strides.
            v8 = v8p.tile([128, N_BLOCKS, 2, 2, 2, 2, OUT_PER_CORE],
                          mybir.dt.float8e4)
            # fp8 w staging for all four blocks (DMA'd once in the prelude)
            wst = wsp.tile([128, N_BLOCKS, 2, 2, 2, 2, OUT_PER_CORE],
                           mybir.dt.float8e4)
            t1f = t1p.tile([128, N_BLOCKS, 2, 2, 2, OUT_PER_CORE],
                           mybir.dt.bfloat16)
            t2f = t2p.tile([128, N_BLOCKS, 2, 2, 2, OUT_PER_CORE],
                           mybir.dt.bfloat16)

            # DMA prelude.  The scalar (ACT) queue must stay almost empty:
            # its sequencer blocks all later ACT compute until queued DMA
            # wire time completes.  So: scalar gets only 4 small w0 chunks;
            # sync (SP has no compute) carries w1-3 interleaved with x
            # group 0; x group 1 prefetches via the Pool SWDGE queue.
            xg_pre = {}
            for g in (0, 1):
                xg_pre[g] = xin.tile([128, K_CHUNKS, 2, G_TOK],
                                     mybir.dt.float8e4, name=f"xg{g}",
                                     tag="xg")
            def w_dma(blk):
                nc.sync.dma_start(wst[:, blk, 0], w_d[blk, :, 0])
                nc.sync.dma_start(wst[:, blk, 1], w_d[blk, :, 1])
            def x_dma(eng, g, blk):
                eng.dma_start(xg_pre[g][:, blk * QR:(blk + 1) * QR, :, :],
                              x_d[g, blk])
            for q in range(QR):
                eng = nc.sync if q % 2 == 0 else nc.scalar
                eng.dma_start(
                    wst[:, 0, (q >> 2) & 1, (q >> 1) & 1, q & 1],
                    w_d[0, :, (q >> 2) & 1, (q >> 1) & 1, q & 1])
            w_dma(1)
            x_dma(nc.sync, 0, 0)
            w_dma(2)
            w_dma(3)
            x_dma(nc.sync, 0, 1)
            x_dma(nc.sync, 0, 2)
            x_dma(nc.sync, 0, 3)
            for blk in range(N_BLOCKS):
                x_dma(nc.gpsimd, 1, blk)

            # Stage A per block: T1 = (H128/32) @ (w_hi + w_lo) on PE via
            # DoubleRow, PSUM evac to SBUF bf16 (ACT, last pair on DVE),
            # bit-0 butterfly add(Pool)/sub(DVE), bits 1-2 on DVE, then
            # the e4m3 hi/lo split of V.
            deferred_lo1 = []
            for blk in range(N_BLOCKS):
                t1 = t1f[:, blk]
                t2 = t2f[:, blk]
                vb = t1   # bit 2 ping-pongs back into t1's space
                for qq in range(QR // 2):
                    q2, q1 = qq >> 1, qq & 1
                    accA = ps1.tile([128, OUT_PER_CORE], mybir.dt.float32)
                    accB = ps1.tile([128, OUT_PER_CORE], mybir.dt.float32)
                    nc.tensor.matmul(accA[:], h[:], wst[:, blk, q2, q1, 0],
                                     start=True, stop=True,
                                     perf_mode=mybir.MatmulPerfMode.DoubleRow)
                    nc.tensor.matmul(accB[:], h[:], wst[:, blk, q2, q1, 1],
                                     start=True, stop=True,
                                     perf_mode=mybir.MatmulPerfMode.DoubleRow)
                    # evac into t2 (the w staging already consumed), then
                    # the bit-0 butterfly in SBUF bf16: add on Pool, sub on
                    # DVE (2x 16-bit).  TensorTensor allows at most one PSUM
                    # operand, so the butterfly cannot read PSUM pairs.
                    ea = t2[:, q2, q1, 0, :]
                    eb = t2[:, q2, q1, 1, :]
                    if qq < 3:
                        nc.scalar.copy(ea, accA[:])
                        nc.scalar.copy(eb, accB[:])
                    else:
                        # last pair on DVE: ACT's evac cadence would gate it
                        nc.vector.tensor_copy(out=ea, in_=accA[:])
                        nc.vector.tensor_copy(out=eb, in_=accB[:])
                    if qq % 2 == 0:
                        nc.gpsimd.tensor_add(t1[:, q2, q1, 0, :], ea, eb)
                    else:
                        # alternate the adds onto DVE: Pool's 1.1us/add
                        # otherwise serializes the block's bit-0 phase
                        nc.vector.tensor_tensor(t1[:, q2, q1, 0, :], ea, eb,
                                                mybir.AluOpType.add)
                    nc.vector.tensor_tensor(t1[:, q2, q1, 1, :], ea, eb,
                                            mybir.AluOpType.subtract)
                # bits 1-2 fully on DVE (2x 16-bit mode; Pool's software
                # ALU is 0.42-efficiency and would gate the chain).  bit 1
                # runs per q2-half so its first half overlaps the second
                # half's matmuls.
                for q2 in (0, 1):
                    a = t1[:, q2, 0, :, :]
                    b = t1[:, q2, 1, :, :]
                    nc.vector.tensor_tensor(t2[:, q2, 0, :, :], a, b,
                                            mybir.AluOpType.add)
                    nc.vector.tensor_tensor(t2[:, q2, 1, :, :], a, b,
                                            mybir.AluOpType.subtract)
                a = t2[:, 0, :, :, :]
                b = t2[:, 1, :, :, :]
                nc.vector.tensor_tensor(vb[:, 0, :, :, :], a, b,
                                        mybir.AluOpType.add)
                nc.vector.tensor_tensor(vb[:, 1, :, :, :], a, b,
                                        mybir.AluOpType.subtract)
                # e4m3 split in q2-halves.  q2=0 (the first stage-2 pairs of
                # the block) takes the fast path: hi on ACT, lo on DVE.  The
                # q2=1 half offloads lo to Pool as add(vb, -hi) with the
                # negated-hi cast on ACT, keeping DVE's block cadence down.
                for q2 in (0, 1):
                    vh = vb[:, q2, :, :, :]
                    hi = v8[:, blk, q2, :, :, 1, :]
                    lo = v8[:, blk, q2, :, :, 0, :]
                    # hi in q1-quarters: the first I_mains of the block
                    # unlock one ACT-op earlier
                    nc.scalar.copy(hi[:, 0, :, :], vh[:, 0, :, :])
                    nc.scalar.copy(hi[:, 1, :, :], vh[:, 1, :, :])
                    if q2 == 1 and all(blk * QR + 4 + i in DROP_CORR
                                       for i in range(4)):
                        continue   # correction-dropped: lo never read
                    if q2 == 0:
                        nc.vector.tensor_tensor(lo, vh, hi,
                                                mybir.AluOpType.subtract)
                    else:
                        # deferred below: keeps Pool off the hi-delivery
                        # critical path during stage A
                        deferred_lo1.append((blk, q2, vh, hi, lo))

            # deferred q2=1 lo splits via negh(ACT)+add(Pool)
            for blk, q2, vh, hi, lo in deferred_lo1:
                nh = nhp.tile([128, 2, 2, OUT_PER_CORE],
                              mybir.dt.float8e4,
                              name=f"nh{blk}", tag="nh")
                nc.scalar.mul(nh[:], vh, -1.0)
                nc.gpsimd.tensor_add(lo, vh, nh[:])

            # --- stage-2 emission machinery ---------------------------
            # The PE sequencer executes its stream in order, so the emission
            # order IS the execution order.  Accumulators live on 6 rotating
            # PSUM banks; per-acc start/stop flags are tracked explicitly so
            # blocks of different groups can interleave.
            group_accs = {}
            group_xg = {}
            started = {}
            done_cnt = {}

            def get_group(g):
                if g not in group_accs:
                    if g in xg_pre:
                        xg = xg_pre[g]
                    else:
                        xg = xin.tile([128, K_CHUNKS, 2, G_TOK],
                                      mybir.dt.float8e4, name=f"xg{g}",
                                      tag="xg")
                        eng = (nc.sync if (g == 3 or g % 2 == 0)
                               else nc.gpsimd)
                        for blk in range(N_BLOCKS):
                            eng.dma_start(
                                xg[:, blk * QR:(blk + 1) * QR, :, :],
                                x_d[g, blk])
                    group_xg[g] = xg
                    group_accs[g] = [
                        ps2.tile([128, OUT_PER_CORE], mybir.dt.float32,
                                 name=f"acc{g}_{ml}",
                                 tag=f"acc{(g * G_M + ml) % 6}")
                        for ml in range(G_M)]
                return group_xg[g], group_accs[g]

            def i_main(g, p, ml):
                xg, accs = get_group(g)
                pb, pq2, pq1 = p >> 2, (p >> 1) & 1, p & 1
                msl = slice(ml * 128, (ml + 1) * 128)
                st = not started.get((g, ml), False)
                started[(g, ml)] = True
                done_cnt[(g, ml)] = done_cnt.get((g, ml), 0) + 1
                nc.tensor.matmul(
                    accs[ml][:], xg[:, 2 * p:2 * p + 2, 0, msl],
                    v8[:, pb, pq2, pq1, :, 1, :], start=st, stop=False,
                    perf_mode=mybir.MatmulPerfMode.DoubleRow)

            def i_corr(g, k, ml):
                xg, accs = get_group(g)
                msl = slice(ml * 128, (ml + 1) * 128)
                st = not started.get((g, ml), False)
                started[(g, ml)] = True
                n = done_cnt.get((g, ml), 0) + 1
                done_cnt[(g, ml)] = n
                nc.tensor.matmul(
                    accs[ml][:], xg[:, k, :, msl],
                    v8[:, k >> 3, (k >> 2) & 1, (k >> 1) & 1, k & 1, :, :],
                    start=st, stop=(n == N_ACC_INSTR),
                    perf_mode=mybir.MatmulPerfMode.DoubleRow)

            def emit_block(g, blk, mls, part="all"):
                # mains first: they need only the hi half of the V split.
                # part: "all" | "lo0" (mains + q2=0 corrs) | "hi1" (q2=1
                # corrs only, which wait on the slow Pool lo1 split)
                if part in ("all", "lo0"):
                    for p in range(blk * 4, blk * 4 + 4):
                        for ml in mls:
                            i_main(g, p, ml)
                k0, k1 = blk * QR, (blk + 1) * QR
                if part == "lo0":
                    k1 = blk * QR + 4
                elif part == "hi1":
                    k0 = blk * QR + 4
                for k in range(k0, k1):
                    if k in DROP_CORR:
                        continue
                    for ml in mls:
                        i_corr(g, k, ml)

            group_ot = {}

            def emit_evac(g, mls, last_g=False):
                if g not in group_ot:
                    group_ot[g] = ost.tile([128, G_M, OUT_PER_CORE],
                                           mybir.dt.bfloat16,
                                           name=f"ot{g}", tag="ot")
                ot = group_ot[g]
                for ml in mls:
                    nc.scalar.mul(ot[:, ml, :], group_accs[g][ml][:],
                                  DESCALE)
                    if last_g:
                        nc.scalar.dma_start(o_d[g, :, ml, :], ot[:, ml, :])

            # --- prologue: hand-interleaved so the PE never heads-of-line
            # blocks on a V block that stage A has not finished yet -------
            ALL = list(range(G_M))
            emit_block(0, 0, ALL)
            emit_block(0, 1, ALL)
            emit_block(1, 0, [0, 1])
            emit_block(0, 2, ALL)
            emit_block(1, 1, [0, 1])
            emit_block(1, 2, [0, 1])
            emit_block(0, 3, ALL)           # g0 closes here
            emit_block(1, 3, [0, 1])
            emit_evac(0, ALL)
            nc.gpsimd.dma_start(o_d[0], group_ot[0][:])
            for blk in range(N_BLOCKS):
                emit_block(1, blk, [2, 3])
            emit_evac(1, ALL)
            nc.gpsimd.dma_start(o_d[1], group_ot[1][:])

            # --- steady state -----------------------------------------
            for g in range(2, N_GROUPS):
                last_g = g == N_GROUPS - 1
                if last_g:
                    # ml-major: each accumulator closes early so the final
                    # evac + out-DMA chain pipelines behind the remaining
                    # matmuls instead of serializing at the very end
                    for ml in range(G_M):
                        for blk in range(N_BLOCKS):
                            emit_block(g, blk, [ml])
                        emit_evac(g, [ml], last_g=True)
                else:
                    for blk in range(N_BLOCKS):
                        emit_block(g, blk, ALL)
                    emit_evac(g, ALL)
                    nc.gpsimd.dma_start(o_d[g], group_ot[g][:])

    nc.compile()
    return nc


def _get_program():
    global _PROGRAM
    if _PROGRAM is None:
        _PROGRAM = _build_program()
    return _PROGRAM


def _prep_inputs(input, weight):
    x = np.asarray(input, dtype=np.float32).reshape(TOK, D) * SX
    x_hi = x.astype(E4M3)
    x_lo = (x - x_hi.astype(np.float32)).astype(E4M3)
    # [g, blk, p, q, t] from [tok, d]
    def lay(a):
        return a.reshape(N_GROUPS, G_TOK, N_BLOCKS, QR, 128).transpose(
            0, 2, 4, 3, 1)
    xg = np.ascontiguousarray(
        np.stack([lay(x_hi), lay(x_lo)], axis=4))  # [g, blk, p, q, 2, t]

    w = np.asarray(weight, dtype=np.float32) * SV
    # h pair: H/32 entries are +-2^-5, exact in e4m3; duplicated so the
    # DoubleRow stationary contracts w_hi and w_lo against the same H
    h1 = _h128_table().astype(np.float32).astype(E4M3)
    h = np.ascontiguousarray(np.stack([h1, h1], axis=1))  # [128, 2, 128]
    in_maps = []
    for c in range(N_CORES):
        wsl = w[c * OUT_PER_CORE:(c + 1) * OUT_PER_CORE, :]  # [512, 4096]
        wq = np.ascontiguousarray(
            wsl.T.reshape(N_BLOCKS, QR, 128, OUT_PER_CORE).transpose(0, 2, 1, 3)
        )  # [blk, p, q, o] fp32
        whi = wq.astype(E4M3)
        wlo = (wq - whi.astype(np.float32)).astype(E4M3)
        wt = np.ascontiguousarray(
            np.stack([whi, wlo], axis=3)  # [blk, p, q, hl, o]
        ).reshape(N_BLOCKS, 128, 2, 2, 2, 2, OUT_PER_CORE)
        in_maps.append({"xg8": xg, "wt": wt, "h": h})
    return in_maps


def kernel(input, weight):
    import time as _time

    nc = _get_program()
    in_maps = _prep_inputs(input, weight)
    # The axon-side XLA compile of the bass_exec custom call is
    # intermittently flaky (CallFunctionObjArgs INTERNAL error) on first
    # compile in a fresh process; a clean retry re-lowers and succeeds.
    last_exc = None
    for attempt in range(3):
        try:
            res = run_bass_kernel_spmd(nc, in_maps, list(range(N_CORES)))
            break
        except Exception as exc:  # noqa: BLE001 - retry transient compile/exec
            # Also rides out a stale device wedge (NRT_EXEC_UNIT_UNRECOVERABLE),
            # which clears on a ~1-2 minute timescale.
            last_exc = exc
            _time.sleep(30.0 * (attempt + 1))
    else:
        raise last_exc
    # out[g, t, ml, o] -> [tok, o]
    parts = [res.results[c]["out"].astype(np.float32).transpose(0, 2, 1, 3)
             .reshape(TOK, OUT_PER_CORE) for c in range(N_CORES)]
    out = np.concatenate(parts, axis=1).reshape(B, S, D)
    return np.ascontiguousarray(out, dtype=np.float32)
